# revision 76
# baseline (speedup 1.0000x reference)
"""Multi-head attention kernel for Trainium2 (Bass/Tile), 8 NeuronCores.

Problem: nn_MultiHeadAttention
  x [8, 1024, 1024] f32, w_qkv [1024, 3072], b_qkv [3072],
  w_proj [1024, 1024], b_proj [1024]  ->  out [8, 1024, 1024]

  qkv = x @ w_qkv + b_qkv ; split (h, d, 3) interleaved on last dim
  score = q k^T per (b, h);  att = softmax(score, -1) / sqrt(1024)
  out = (att @ v) reshaped @ w_proj + b_proj

Sharding: data-parallel over batch. Each of the 8 cores runs the full
MHA for one batch element; no collectives. Host pre-transposes x and
pre-splits w_qkv so the device program is pure matmul + softmax.

Perf design (measured 686us -> 306us on HW, rel err 3.5e-3):
  - all matmul operands are 2-byte (fp16 for x/w/qT/kT/ao/wp, bf16 for
    E and v): full-rate PE streams, half-size weight loads, half DMA.
    fp32 PSUM accumulation throughout.
  - attention is head-serial: scores for one (head, k-tile) fill a
    [128, TOK] PSUM tile (two 512-col matmuls, PSUM bank limit) and
    take ONE wide exp on ACT - fewer, larger ACT instructions. attV
    trails the exp by one k-tile so it never waits on exp latency.
    PSUM: 2x scores tiles (4 banks) + 1 QK/bc bank + 3 O' banks.
  - softmax denominator rides as a 65th "ones" column of v; normalize
    uses reciprocal_approx_fast (SBUF-staged - it misreads PSUM on HW),
    a PE outer-product broadcast, and one DVE multiply, deferred by one
    head so the PE never waits on the reciprocal chain.
  - QK projection for pair p+1 and the wp prefetch are interleaved
    into the attention loop; x is split across the two hardware DMA
    rings (sync + scalar/ACT). gpsimd's software DGE corrupts strided
    DMA patterns and must only carry contiguous transfers, and its
    tensor ops (no PSUM access, ~4ns/elem) are avoided.

Device-side math per core (layouts chosen so no on-device transpose is
ever needed):
  v  = x wv + bv     [tok, (h,d)]  + ones-column per head -> v_aug
  qT = (x wq)^T + bq [(h,d), tok]
  kT = (x wk)^T + bk
  per head: S^T[k,q] = kT-slice.T @ qT-slice; E = exp(S^T) (bf16)
            O'^T[0:64,q], O'^T[64,q] = sum_k E   (v_aug ones column)
            ao^T = O'[0:64] * (att_scale / O'[64])
  out = ao^T.T @ wp + bp   (biases via ones outer-product matmuls)
"""

import os

os.environ.setdefault("MYCRO_LOCAL_CACHE", "1")

import numpy as np

import concourse.bass as bass
import concourse.tile as tile
from concourse import bacc, mybir

P = 128
DH = 64  # head dim
F32 = mybir.dt.float32
F16 = mybir.dt.float16
BF16 = mybir.dt.bfloat16

# full-problem constants
B_FULL = 8
TOK_FULL = 1024
D_FULL = 1024
H_FULL = 16
ATT_SCALE_FULL = 1.0 / 32.0  # 1/sqrt(1024), applied after softmax
N_CORES = 8


def _chunks(total, step=512):
    return [(s, min(step, total - s)) for s in range(0, total, step)]


def build(nc, TOK, D, H, att_scale):
    """Emit the one-core MHA program (one batch element).

    DRAM inputs (host pre-laid-out, fp16 unless noted):
      x        [P, KT*TOK]   [p, kt, t] = x[t, kt*P + p]   (x^T, kt-tiled)
      wq/wk/wv/wp [P, KT*D]  [p, kt, n] = w[kt*P + p, n]
      bq/bk    [P, NPAIR] f32  [p, m] = b[m*P + p]
      bv/bp    [1, D]
    Output: out [TOK, D] f32
    """
    assert D == H * DH and D % P == 0 and TOK % P == 0 and H % 2 == 0
    KT = D // P       # contraction tiles over the model dim
    MT = TOK // P     # token (and k) tiles
    NPAIR = H // 2    # head pairs (== D // P)
    VW = H * (DH + 1)  # v_aug row width: per head [v | 1]
    EXP = mybir.ActivationFunctionType.Exp
    QCH = _chunks(TOK, 512)   # q chunks (PSUM bank = 512 fp32)
    DCH = _chunks(D, 512)     # model-dim chunks

    x_d = nc.dram_tensor("x", [P, KT * TOK], F16, kind="ExternalInput")
    w_d = {}
    for nm in ("wq", "wk", "wv", "wp"):
        w_d[nm] = nc.dram_tensor(nm, [P, KT * D], F16, kind="ExternalInput")

    bq_d = nc.dram_tensor("bq", [P, NPAIR], F32, kind="ExternalInput")
    bk_d = nc.dram_tensor("bk", [P, NPAIR], F32, kind="ExternalInput")
    bv_d = nc.dram_tensor("bv", [1, D], F16, kind="ExternalInput")
    bp_d = nc.dram_tensor("bp", [1, D], F16, kind="ExternalInput")
    out_d = nc.dram_tensor("out", [TOK, D], F32, kind="ExternalOutput")

    with tile.TileContext(nc) as tc:
        with (
            tc.tile_pool(name="sing", bufs=1) as sing,
            tc.tile_pool(name="psS", bufs=2, space="PSUM") as psS,
            tc.tile_pool(name="psQ", bufs=1, space="PSUM") as psQ,
            tc.tile_pool(name="psO", bufs=3, space="PSUM") as psO,
            tc.tile_pool(name="ebuf", bufs=6) as ebuf,
            tc.tile_pool(name="araw", bufs=6) as araw,
            tc.tile_pool(name="rpool", bufs=2) as rpool,
            tc.tile_pool(name="rpool16", bufs=2) as rpool16,
            tc.tile_pool(name="wqk", bufs=4) as wqk,
            tc.tile_pool(name="wpp", bufs=2) as wpp,
            tc.tile_pool(name="outp", bufs=2) as outp,
        ):
            # ---------------- persistent SBUF ----------------
            # memset targets f32; 16-bit constants made via cast copies
            cst_sb = sing.tile([1, P + DH], F32, tag="cst")
            nc.vector.memset(cst_sb[:, 0:P], 1.0)
            nc.vector.memset(cst_sb[:, P : P + DH], att_scale)
            ones_sb = sing.tile([1, P], F16, tag="ones")
            nc.vector.tensor_copy(out=ones_sb, in_=cst_sb[:, 0:P])
            scl_sb = sing.tile([1, DH], BF16, tag="scl")
            nc.vector.tensor_copy(out=scl_sb, in_=cst_sb[:, P : P + DH])
            vones_sb = sing.tile([P, MT * H], F32, tag="vones")
            nc.vector.memset(vones_sb, 1.0)

            # pair-0 qk weights first (small, they gate the first matmuls
            # together with x's first piece), then x in kt-granular pieces
            # alternating over the two hardware DMA rings (sync + scalar):
            # the first QK matmuls start as soon as piece 0 lands. gpsimd's
            # software DGE corrupts strided patterns - keep it off DMA.
            qk_tiles0 = {}
            for wi, wname in enumerate(("wq", "wk")):
                w_sb = wqk.tile([P, KT, P], F16, name="w_sb", tag="w" + wname)
                (nc.sync if wi == 0 else nc.scalar).dma_start(
                    out=w_sb[:, :, :],
                    in_=w_d[wname][:, :].rearrange("p (kt n) -> p kt n", n=D)[
                        :, :, 0:P
                    ],
                )
                qk_tiles0[wname] = w_sb

            XS = max(1, KT // 4)  # kts per x piece
            NXP = (KT + XS - 1) // XS
            x_t = []
            for j in range(NXP):
                k0, k1 = j * XS, min((j + 1) * XS, KT)
                xt = sing.tile(
                    [P, (k1 - k0) * TOK], F16, name="x_t", tag=f"x{j}"
                )
                (nc.sync if j % 2 == 0 else nc.scalar).dma_start(
                    out=xt, in_=x_d[:, k0 * TOK : k1 * TOK]
                )
                x_t.append(xt[:, :].rearrange("p (kt t) -> p kt t", t=TOK))

            def x3(kt):
                return x_t[kt // XS][:, kt % XS, :]

            bq_sb = sing.tile([P, NPAIR], F32, tag="bq")
            nc.sync.dma_start(out=bq_sb, in_=bq_d[:, :])
            bk_sb = sing.tile([P, NPAIR], F32, tag="bk")
            nc.sync.dma_start(out=bk_sb, in_=bk_d[:, :])
            bv_sb = sing.tile([1, D], F16, tag="bv")
            nc.sync.dma_start(out=bv_sb, in_=bv_d[:, :])
            bp_sb = sing.tile([1, D], F16, tag="bp")
            nc.sync.dma_start(out=bp_sb, in_=bp_d[:, :])

            v_sb = sing.tile([P, MT, VW], BF16, tag="v")     # v_aug
            # ones columns (denominator accumulators), cast f32->bf16
            nc.vector.tensor_copy(
                out=v_sb[:, :, :]
                .rearrange("p m (h e) -> p m h e", e=DH + 1)[:, :, :, DH],
                in_=vones_sb[:, :].rearrange("p (m h) -> p m h", h=H),
            )
            qT_sb = sing.tile([P, NPAIR, TOK], F16, tag="qT")
            kT_sb = sing.tile([P, NPAIR, TOK], F16, tag="kT")
            ao_sb = sing.tile([P, NPAIR, TOK], F16, tag="ao")  # attout^T

            # ---------------- QK projection helpers ----------------
            # part 0/1 = wq chunks, part 2/3 = wk chunks (when NCH==2).
            NCH = len(QCH)

            def emit_qk_dma2(pp):
                tiles = {}
                for wname in ("wq", "wk"):
                    w_sb = wqk.tile([P, KT, P], F16, name="w_sb", tag="w" + wname)
                    src = w_d[wname][:, :].rearrange("p (kt n) -> p kt n", n=D)[
                        :, :, pp * P : (pp + 1) * P
                    ]
                    nc.sync.dma_start(out=w_sb[:, :, :], in_=src)
                    tiles[wname] = w_sb
                return tiles

            def emit_qk_part2(pp, tiles, part):
                wname, dst_sb, b_sb = (
                    ("wq", qT_sb, bq_sb) if part < NCH else ("wk", kT_sb, bk_sb)
                )
                c0, cw = QCH[part % NCH]
                w_sb = tiles[wname]
                ps_q = psQ.tile([P, 512], F32, name="ps_q", tag="psQ")
                for kt in range(KT):
                    nc.tensor.matmul(
                        ps_q[:, 0:cw],
                        lhsT=w_sb[:, kt, :],
                        rhs=x3(kt)[:, c0 : c0 + cw],
                        start=(kt == 0),
                        stop=(kt == KT - 1),
                    )
                # bias-add eviction on ACT (Identity shares exp's table, so
                # no table swaps); keeps the Vector queue free for the
                # PSUM-releasing attention eviction copies
                nc.scalar.activation(
                    out=dst_sb[:, pp, c0 : c0 + cw],
                    in_=ps_q[:, 0:cw],
                    func=mybir.ActivationFunctionType.Identity,
                    bias=b_sb[:, pp : pp + 1],
                )

            # pair-0 QK first: gated only by x + a small weight slice, it
            # gives the PE work while the (larger) wv transfer is in flight
            for part in range(2 * NCH):
                emit_qk_part2(0, qk_tiles0, part)

            # ---------------- V phase: v = x wv + bv (natural) ----
            with tc.tile_pool(name="wvp", bufs=2) as wvp:
                for ci, (c0, cw) in enumerate(DCH):
                    wv_sb = wvp.tile([P, KT, 512], F16, tag="wv")
                    wv_src = w_d["wv"][:, :].rearrange(
                        "p (kt n) -> p kt n", n=D
                    )[:, :, c0 : c0 + cw]
                    if ci == 0:
                        # first chunk gates the V phase: halves on both
                        # hardware rings so it lands right as the QK
                        # bridge work runs dry
                        h = cw // 2
                        nc.scalar.dma_start(
                            out=wv_sb[:, :, 0:h], in_=wv_src[:, :, 0:h]
                        )
                        nc.sync.dma_start(
                            out=wv_sb[:, :, h:cw], in_=wv_src[:, :, h:]
                        )
                    else:
                        nc.sync.dma_start(out=wv_sb[:, :, 0:cw], in_=wv_src)
                    for mt in range(MT):
                        ps_v = psS.tile([P, 512], F32, tag="psS")
                        for kt in range(KT):
                            nc.tensor.matmul(
                                ps_v[:, 0:cw],
                                lhsT=x3(kt)[:, mt * P : (mt + 1) * P],
                                rhs=wv_sb[:, kt, 0:cw],
                                start=(kt == 0),
                                stop=False,
                            )
                        # + bv by ones outer-product
                        nc.tensor.matmul(
                            ps_v[:, 0:cw],
                            lhsT=ones_sb[0:1, 0:P],
                            rhs=bv_sb[0:1, c0 : c0 + cw],
                            start=False,
                            stop=True,
                        )
                        # scatter heads into v_aug (65-stride)
                        nh = cw // DH
                        h0 = c0 // DH
                        nc.vector.tensor_copy(
                            out=v_sb[:, mt, :]
                            .rearrange("p (h e) -> p h e", e=DH + 1)[
                                :, h0 : h0 + nh, 0:DH
                            ],
                            in_=ps_v[:, 0:cw].rearrange(
                                "p (h d) -> p h d", d=DH
                            ),
                        )

            # prefetch projection weights so they land long before the
            # projection phase starts
            wp_tiles = []
            for c0, cw in DCH:
                wp_sb = wpp.tile([P, KT, 512], F16, name="wp_sb", tag="wp")
                nc.sync.dma_start(
                    out=wp_sb[:, :, 0:cw],
                    in_=w_d["wp"][:, :]
                    .rearrange("p (kt n) -> p kt n", n=D)[:, :, c0 : c0 + cw],
                )
                wp_tiles.append(wp_sb)

            # ---------------- attention, head-serial ----------------
            # Scores for a (head, k-tile) fill one [128, TOK] PSUM tile
            # (two 512-col matmuls) and take ONE wide exp — fewer, larger
            # ACT instructions pace the whole attention phase. attV trails
            # one k-tile behind; normalize for head h is deferred into
            # head h+1's loop so the PE never waits on the recip chain.
            pending_norm = []

            def emit_normalize(hh, rinv16, ar):
                p, hoff = hh // 2, hh % 2
                for ci, (c0, cw) in enumerate(QCH):
                    bc = psQ.tile([P, 512], F32, name="bc", tag="psQ")
                    nc.tensor.matmul(
                        bc[0:DH, 0:cw],
                        lhsT=scl_sb[0:1, 0:DH],
                        rhs=rinv16[0:1, c0 : c0 + cw],
                        start=True,
                        stop=True,
                    )
                    nc.vector.tensor_mul(
                        out=ao_sb[
                            hoff * DH : (hoff + 1) * DH, p, c0 : c0 + cw
                        ],
                        in0=ar[0:DH, c0 : c0 + cw],
                        in1=bc[0:DH, 0:cw],
                    )

            norm_kb = min(3, MT - 1)
            qk_kbs = [min(2 + 3 * i, MT - 1) for i in range(NCH)]
            for hh in range(H):
                p, hoff = hh // 2, hh % 2
                base = hoff * DH
                o_t = [
                    psO.tile([DH + 1, 512], F32, name="o_t", tag="psO")
                    for _ in QCH
                ]
                emit_next_qk = p + 1 < NPAIR
                if hoff == 0 and emit_next_qk:
                    next_tiles = emit_qk_dma2(p + 1)

                def emit_attv(kb, ee):
                    for ci, (c0, cw) in enumerate(QCH):
                        nc.tensor.matmul(
                            o_t[ci][:, 0:cw],
                            lhsT=v_sb[
                                :, kb, hh * (DH + 1) : (hh + 1) * (DH + 1)
                            ],
                            rhs=ee[:, c0 : c0 + cw],
                            start=(kb == 0),
                            stop=(kb == MT - 1),
                            skip_group_check=True,
                        )

                # attV trails the scores/exp by TWO k-tiles: its exp input
                # is then always long finished, so the in-order PE queue
                # never stalls mid-block and ACT paces at its own rate
                AVD = min(2, MT - 1)
                e_q = []
                for kb in range(MT):
                    ps = psS.tile([P, TOK], F32, tag="psS")
                    for ci, (c0, cw) in enumerate(QCH):
                        nc.tensor.matmul(
                            ps[:, c0 : c0 + cw],
                            lhsT=kT_sb[
                                base : base + DH, p, kb * P : (kb + 1) * P
                            ],
                            rhs=qT_sb[base : base + DH, p, c0 : c0 + cw],
                            start=True,
                            stop=True,
                        )
                    ee = ebuf.tile([P, TOK], BF16, tag="E")
                    nc.scalar.activation(out=ee[:, :], in_=ps[:, :], func=EXP)
                    e_q.append(ee)
                    if kb >= AVD:
                        emit_attv(kb - AVD, e_q[kb - AVD])
                    # keep PE fed: next pair's QK between attention k-tiles
                    if emit_next_qk and kb in qk_kbs:
                        emit_qk_part2(
                            p + 1, next_tiles, hoff * NCH + qk_kbs.index(kb)
                        )
                    # previous head's normalize, deps long since ready
                    if kb == norm_kb and pending_norm:
                        emit_normalize(*pending_norm.pop(0))
                for kb in range(MT - AVD, MT):
                    emit_attv(kb, e_q[kb])

                # epilogue: evict O' (bf16, incl. denom row) freeing PSUM;
                # denominators to SBUF fp32 (reciprocal_approx_fast needs
                # fp32 and misreads PSUM); recip + bf16 cast on DVE.
                den = rpool.tile([1, TOK], F32, tag="den")
                rinv = rpool.tile([1, TOK], F32, tag="r")
                rinv16 = rpool16.tile([1, TOK], BF16, tag="r16")
                ar = araw.tile([DH + 1, TOK], BF16, tag="ar")
                last = hh == H - 1
                if last:
                    # last head: denominators first so the recip chain (the
                    # serial tail before the projection) starts asap
                    for ci, (c0, cw) in enumerate(QCH):
                        nc.vector.tensor_copy(
                            out=den[0:1, c0 : c0 + cw],
                            in_=o_t[ci][DH : DH + 1, 0:cw],
                        )
                    nc.vector.reciprocal_approx_fast(out=rinv, in_=den)
                    nc.vector.tensor_copy(out=rinv16, in_=rinv)
                for ci, (c0, cw) in enumerate(QCH):
                    nc.vector.tensor_copy(
                        out=ar[:, c0 : c0 + cw], in_=o_t[ci][:, 0:cw]
                    )
                    if not last:
                        nc.vector.tensor_copy(
                            out=den[0:1, c0 : c0 + cw],
                            in_=o_t[ci][DH : DH + 1, 0:cw],
                        )
                if not last:
                    nc.vector.reciprocal_approx_fast(out=rinv, in_=den)
                    nc.vector.tensor_copy(out=rinv16, in_=rinv)
                pending_norm.append((hh, rinv16, ar))

            # ---------------- projection: out = attout wp + bp -------
            def emit_proj_group(ci, mt, ps_p, kts):
                c0, cw = DCH[ci]
                wp_sb = wp_tiles[ci]
                for kt in kts:
                    nc.tensor.matmul(
                        ps_p[:, 0:cw],
                        lhsT=ao_sb[:, kt, mt * P : (mt + 1) * P],
                        rhs=wp_sb[:, kt, 0:cw],
                        start=(kt == 0),
                        stop=False,
                    )
                if kts and kts[-1] != KT - 1:
                    return
                nc.tensor.matmul(
                    ps_p[:, 0:cw],
                    lhsT=ones_sb[0:1, 0:P],
                    rhs=bp_sb[0:1, c0 : c0 + cw],
                    start=False,
                    stop=True,
                )
                # evict on ACT (idle after the last exp) and return the
                # result over both hardware DMA rings alternately
                o_sb = outp.tile([P, 512], F32, name="o_sb", tag="o")
                nc.scalar.copy(out=o_sb[:, 0:cw], in_=ps_p[:, 0:cw])
                (nc.sync if mt % 2 == 0 else nc.scalar).dma_start(
                    out=out_d[mt * P : (mt + 1) * P, c0 : c0 + cw],
                    in_=o_sb[:, 0:cw],
                )

            while pending_norm:
                emit_normalize(*pending_norm.pop(0))
            for ci in range(len(DCH)):
                for mt in range(MT):
                    ps_p = psS.tile([P, 512], F32, name="ps_p", tag="psS")
                    emit_proj_group(ci, mt, ps_p, list(range(KT)))

    return nc


# ---------------------------------------------------------------------------
# host-side layout prep
# ---------------------------------------------------------------------------

def host_prep_shared(w_qkv, b_qkv, w_proj, b_proj, D, H):
    """Split/retile the weights once for all cores."""
    KT = D // P
    NPAIR = H // 2

    def tile_w(w):  # [D, N] -> [P, KT*N] fp16
        N = w.shape[1]
        return np.ascontiguousarray(
            w.reshape(KT, P, N).transpose(1, 0, 2).reshape(P, KT * N)
        ).astype(np.float16)

    wq3 = w_qkv.reshape(D, H, DH, 3)
    out = {
        "wq": tile_w(np.ascontiguousarray(wq3[:, :, :, 0].reshape(D, D))),
        "wk": tile_w(np.ascontiguousarray(wq3[:, :, :, 1].reshape(D, D))),
        "wv": tile_w(np.ascontiguousarray(wq3[:, :, :, 2].reshape(D, D))),
        "wp": tile_w(np.ascontiguousarray(w_proj)),
    }
    b3 = b_qkv.reshape(H, DH, 3)
    bq = np.ascontiguousarray(b3[:, :, 0].reshape(D))
    bk = np.ascontiguousarray(b3[:, :, 1].reshape(D))
    bv = np.ascontiguousarray(b3[:, :, 2].reshape(D))
    out["bq"] = np.ascontiguousarray(bq.reshape(NPAIR, P).T).astype(np.float32)
    out["bk"] = np.ascontiguousarray(bk.reshape(NPAIR, P).T).astype(np.float32)
    out["bv"] = bv.reshape(1, D).astype(np.float16)
    out["bp"] = np.asarray(b_proj, np.float32).reshape(1, D).astype(np.float16)
    return out


def host_prep_x(x_b, TOK, D):
    """One batch element [TOK, D] -> x^T tiled [P, KT*TOK] fp16."""
    KT = D // P
    xT = np.ascontiguousarray(np.asarray(x_b, np.float32).T)  # [D, TOK]
    return np.ascontiguousarray(
        xT.reshape(KT, P, TOK).transpose(1, 0, 2).reshape(P, KT * TOK)
    ).astype(np.float16)


# ---------------------------------------------------------------------------
# entry point
# ---------------------------------------------------------------------------

_BUILT = {}


def _get_nc(TOK, D, H, att_scale):
    key = (TOK, D, H, att_scale)
    if key not in _BUILT:
        nc = bacc.Bacc(
            "TRN2",
            target_bir_lowering=False,
            debug=False,
            dynamic_dma_scratch_size=512,
        )
        build(nc, TOK, D, H, att_scale)
        nc.compile()
        nc.finalize()
        _BUILT[key] = nc
    return _BUILT[key]


def kernel(x, w_qkv, b_qkv, w_proj, b_proj):
    from concourse.bass_utils import run_bass_kernel_spmd

    x = np.asarray(x, np.float32)
    B, TOK, D = x.shape
    H = H_FULL
    shared = host_prep_shared(
        np.asarray(w_qkv, np.float32),
        np.asarray(b_qkv, np.float32),
        np.asarray(w_proj, np.float32),
        np.asarray(b_proj, np.float32),
        D,
        H,
    )
    in_maps = []
    for b in range(B):
        m = dict(shared)
        m["x"] = host_prep_x(x[b], TOK, D)
        in_maps.append(m)

    nc = _get_nc(TOK, D, H, ATT_SCALE_FULL)
    res = run_bass_kernel_spmd(nc, in_maps, list(range(N_CORES)))
    out = np.stack([res.results[b]["out"] for b in range(B)], axis=0)
    return out.astype(np.float32)


# revision 77
# speedup vs baseline: 1.1910x; 1.1910x over previous
"""Multi-head attention kernel for Trainium2 (Bass/Tile), 8 NeuronCores.

Problem: nn_MultiHeadAttention
  x [8, 1024, 1024] f32, w_qkv [1024, 3072], b_qkv [3072],
  w_proj [1024, 1024], b_proj [1024]  ->  out [8, 1024, 1024]

  qkv = x @ w_qkv + b_qkv ; split (h, d, 3) interleaved on last dim
  score = q k^T per (b, h);  att = softmax(score, -1) / sqrt(1024)
  out = (att @ v) reshaped @ w_proj + b_proj

Sharding: data-parallel over batch. Each of the 8 cores runs the full
MHA for one batch element; no collectives. Host pre-transposes x and
pre-splits w_qkv so the device program is pure matmul + softmax.

Perf design (measured 686us -> 306us on HW, rel err 3.5e-3):
  - all matmul operands are 2-byte (fp16 for x/w/qT/kT/ao/wp, bf16 for
    E and v): full-rate PE streams, half-size weight loads, half DMA.
    fp32 PSUM accumulation throughout.
  - attention is head-serial: scores for one (head, k-tile) fill a
    [128, TOK] PSUM tile (two 512-col matmuls, PSUM bank limit) and
    take ONE wide exp on ACT - fewer, larger ACT instructions. attV
    trails the exp by one k-tile so it never waits on exp latency.
    PSUM: 2x scores tiles (4 banks) + 1 QK/bc bank + 3 O' banks.
  - softmax denominator rides as a 65th "ones" column of v; normalize
    uses reciprocal_approx_fast (SBUF-staged - it misreads PSUM on HW),
    a PE outer-product broadcast, and one DVE multiply, deferred by one
    head so the PE never waits on the reciprocal chain.
  - QK projection for pair p+1 and the wp prefetch are interleaved
    into the attention loop; x is split across the two hardware DMA
    rings (sync + scalar/ACT). gpsimd's software DGE corrupts strided
    DMA patterns and must only carry contiguous transfers, and its
    tensor ops (no PSUM access, ~4ns/elem) are avoided.

Device-side math per core (layouts chosen so no on-device transpose is
ever needed):
  v  = x wv + bv     [tok, (h,d)]  + ones-column per head -> v_aug
  qT = (x wq)^T + bq [(h,d), tok]
  kT = (x wk)^T + bk
  per head: S^T[k,q] = kT-slice.T @ qT-slice; E = exp(S^T) (bf16)
            O'^T[0:64,q], O'^T[64,q] = sum_k E   (v_aug ones column)
            ao^T = O'[0:64] * (att_scale / O'[64])
  out = ao^T.T @ wp + bp   (biases via ones outer-product matmuls)
"""

import os

os.environ.setdefault("MYCRO_LOCAL_CACHE", "1")

import numpy as np

import concourse.bass as bass
import concourse.tile as tile
from concourse import bacc, mybir

P = 128
DH = 64  # head dim
F32 = mybir.dt.float32
F16 = mybir.dt.float16
BF16 = mybir.dt.bfloat16

# full-problem constants
B_FULL = 8
TOK_FULL = 1024
D_FULL = 1024
H_FULL = 16
ATT_SCALE_FULL = 1.0 / 32.0  # 1/sqrt(1024), applied after softmax
N_CORES = 8


def _chunks(total, step=512):
    return [(s, min(step, total - s)) for s in range(0, total, step)]


def build(nc, TOK, D, H, att_scale):
    """Emit the one-core MHA program (one batch element).

    DRAM inputs (host pre-laid-out, fp16 unless noted):
      x        [P, KT*TOK]   [p, kt, t] = x[t, kt*P + p]   (x^T, kt-tiled)
      wq/wk/wv/wp [P, KT*D]  [p, kt, n] = w[kt*P + p, n]
      bq/bk    [P, NPAIR] f32  [p, m] = b[m*P + p]
      bv/bp    [1, D]
    Output: out [TOK, D] f32
    """
    assert D == H * DH and D % P == 0 and TOK % P == 0 and H % 2 == 0
    KT = D // P       # contraction tiles over the model dim
    MT = TOK // P     # token (and k) tiles
    NPAIR = H // 2    # head pairs (== D // P)
    VW = H * (DH + 1)  # v_aug row width: per head [v | 1]
    EXP = mybir.ActivationFunctionType.Exp
    QCH = _chunks(TOK, 512)   # q chunks (PSUM bank = 512 fp32)
    DCH = _chunks(D, 512)     # model-dim chunks

    x_d = nc.dram_tensor("x", [P, KT * TOK], F16, kind="ExternalInput")
    w_d = {}
    for nm in ("wq", "wk", "wv", "wp"):
        w_d[nm] = nc.dram_tensor(nm, [P, KT * D], F16, kind="ExternalInput")

    bq_d = nc.dram_tensor("bq", [P, NPAIR], F32, kind="ExternalInput")
    bk_d = nc.dram_tensor("bk", [P, NPAIR], F32, kind="ExternalInput")
    bv_d = nc.dram_tensor("bv", [1, D], F16, kind="ExternalInput")
    bp_d = nc.dram_tensor("bp", [1, D], F16, kind="ExternalInput")
    out_d = nc.dram_tensor("out", [TOK, D], F32, kind="ExternalOutput")

    with tile.TileContext(nc) as tc:
        with (
            tc.tile_pool(name="sing", bufs=1) as sing,
            tc.tile_pool(name="psS", bufs=2, space="PSUM") as psS,
            tc.tile_pool(name="psQ", bufs=1, space="PSUM") as psQ,
            tc.tile_pool(name="psO", bufs=3, space="PSUM") as psO,
            tc.tile_pool(name="ebuf", bufs=6) as ebuf,
            tc.tile_pool(name="araw", bufs=6) as araw,
            tc.tile_pool(name="rpool", bufs=2) as rpool,
            tc.tile_pool(name="rpool16", bufs=2) as rpool16,
            tc.tile_pool(name="wqk", bufs=4) as wqk,
            tc.tile_pool(name="wpp", bufs=2) as wpp,
            tc.tile_pool(name="outp", bufs=2) as outp,
        ):
            # ---------------- persistent SBUF ----------------
            # memset targets f32; 16-bit constants made via cast copies
            cst_sb = sing.tile([1, P + DH], F32, tag="cst")
            nc.vector.memset(cst_sb[:, 0:P], 1.0)
            nc.vector.memset(cst_sb[:, P : P + DH], att_scale)
            ones_sb = sing.tile([1, P], F16, tag="ones")
            nc.vector.tensor_copy(out=ones_sb, in_=cst_sb[:, 0:P])
            scl_sb = sing.tile([1, DH], BF16, tag="scl")
            nc.vector.tensor_copy(out=scl_sb, in_=cst_sb[:, P : P + DH])
            vones_sb = sing.tile([P, MT * H], F32, tag="vones")
            nc.vector.memset(vones_sb, 1.0)

            # pair-0 qk weights first (small, they gate the first matmuls
            # together with x's first piece), then x in kt-granular pieces
            # alternating over the two hardware DMA rings (sync + scalar):
            # the first QK matmuls start as soon as piece 0 lands. gpsimd's
            # software DGE corrupts strided patterns - keep it off DMA.
            qk_tiles0 = {}
            for wi, wname in enumerate(("wq", "wk")):
                w_sb = wqk.tile([P, KT, P], F16, name="w_sb", tag="w" + wname)
                (nc.sync if wi == 0 else nc.scalar).dma_start(
                    out=w_sb[:, :, :],
                    in_=w_d[wname][:, :].rearrange("p (kt n) -> p kt n", n=D)[
                        :, :, 0:P
                    ],
                )
                qk_tiles0[wname] = w_sb

            XS = max(1, KT // 4)  # kts per x piece
            NXP = (KT + XS - 1) // XS
            x_t = []
            for j in range(NXP):
                k0, k1 = j * XS, min((j + 1) * XS, KT)
                xt = sing.tile(
                    [P, (k1 - k0) * TOK], F16, name="x_t", tag=f"x{j}"
                )
                (nc.sync if j % 2 == 0 else nc.scalar).dma_start(
                    out=xt, in_=x_d[:, k0 * TOK : k1 * TOK]
                )
                x_t.append(xt[:, :].rearrange("p (kt t) -> p kt t", t=TOK))

            def x3(kt):
                return x_t[kt // XS][:, kt % XS, :]

            bq_sb = sing.tile([P, NPAIR], F32, tag="bq")
            nc.sync.dma_start(out=bq_sb, in_=bq_d[:, :])
            bk_sb = sing.tile([P, NPAIR], F32, tag="bk")
            nc.sync.dma_start(out=bk_sb, in_=bk_d[:, :])
            bv_sb = sing.tile([1, D], F16, tag="bv")
            nc.sync.dma_start(out=bv_sb, in_=bv_d[:, :])
            bp_sb = sing.tile([1, D], F16, tag="bp")
            nc.sync.dma_start(out=bp_sb, in_=bp_d[:, :])

            v_sb = sing.tile([P, MT, VW], BF16, tag="v")     # v_aug
            # ones columns (denominator accumulators), cast f32->bf16
            nc.vector.tensor_copy(
                out=v_sb[:, :, :]
                .rearrange("p m (h e) -> p m h e", e=DH + 1)[:, :, :, DH],
                in_=vones_sb[:, :].rearrange("p (m h) -> p m h", h=H),
            )
            qT_sb = sing.tile([P, NPAIR, TOK], F16, tag="qT")
            kT_sb = sing.tile([P, NPAIR, TOK], F16, tag="kT")
            ao_sb = sing.tile([P, NPAIR, TOK], F16, tag="ao")  # attout^T

            # ---------------- QK projection helpers ----------------
            # part 0/1 = wq chunks, part 2/3 = wk chunks (when NCH==2).
            NCH = len(QCH)

            def emit_qk_dma2(pp):
                tiles = {}
                for wname in ("wq", "wk"):
                    w_sb = wqk.tile([P, KT, P], F16, name="w_sb", tag="w" + wname)
                    src = w_d[wname][:, :].rearrange("p (kt n) -> p kt n", n=D)[
                        :, :, pp * P : (pp + 1) * P
                    ]
                    nc.sync.dma_start(out=w_sb[:, :, :], in_=src)
                    tiles[wname] = w_sb
                return tiles

            def emit_qk_part2(pp, tiles, part):
                wname, dst_sb, b_sb = (
                    ("wq", qT_sb, bq_sb) if part < NCH else ("wk", kT_sb, bk_sb)
                )
                c0, cw = QCH[part % NCH]
                w_sb = tiles[wname]
                ps_q = psQ.tile([P, 512], F32, name="ps_q", tag="psQ")
                for kt in range(KT):
                    nc.tensor.matmul(
                        ps_q[:, 0:cw],
                        lhsT=w_sb[:, kt, :],
                        rhs=x3(kt)[:, c0 : c0 + cw],
                        start=(kt == 0),
                        stop=(kt == KT - 1),
                    )
                # bias-add eviction on ACT (Identity shares exp's table, so
                # no table swaps); keeps the Vector queue free for the
                # PSUM-releasing attention eviction copies
                nc.scalar.activation(
                    out=dst_sb[:, pp, c0 : c0 + cw],
                    in_=ps_q[:, 0:cw],
                    func=mybir.ActivationFunctionType.Identity,
                    bias=b_sb[:, pp : pp + 1],
                )

            # pair-0 QK first: gated only by x + a small weight slice, it
            # gives the PE work while the (larger) wv transfer is in flight
            for part in range(2 * NCH):
                emit_qk_part2(0, qk_tiles0, part)

            # ---------------- V phase: v = x wv + bv (natural) ----
            with tc.tile_pool(name="wvp", bufs=2) as wvp:
                for ci, (c0, cw) in enumerate(DCH):
                    wv_sb = wvp.tile([P, KT, 512], F16, tag="wv")
                    (nc.scalar if ci == 0 else nc.sync).dma_start(
                        out=wv_sb[:, :, 0:cw],
                        in_=w_d["wv"][:, :]
                        .rearrange("p (kt n) -> p kt n", n=D)[:, :, c0 : c0 + cw],
                    )
                    for mt in range(MT):
                        ps_v = psS.tile([P, 512], F32, tag="psS")
                        for kt in range(KT):
                            nc.tensor.matmul(
                                ps_v[:, 0:cw],
                                lhsT=x3(kt)[:, mt * P : (mt + 1) * P],
                                rhs=wv_sb[:, kt, 0:cw],
                                start=(kt == 0),
                                stop=False,
                            )
                        # + bv by ones outer-product
                        nc.tensor.matmul(
                            ps_v[:, 0:cw],
                            lhsT=ones_sb[0:1, 0:P],
                            rhs=bv_sb[0:1, c0 : c0 + cw],
                            start=False,
                            stop=True,
                        )
                        # scatter heads into v_aug (65-stride)
                        nh = cw // DH
                        h0 = c0 // DH
                        nc.vector.tensor_copy(
                            out=v_sb[:, mt, :]
                            .rearrange("p (h e) -> p h e", e=DH + 1)[
                                :, h0 : h0 + nh, 0:DH
                            ],
                            in_=ps_v[:, 0:cw].rearrange(
                                "p (h d) -> p h d", d=DH
                            ),
                        )

            # prefetch projection weights so they land long before the
            # projection phase starts
            wp_tiles = []
            for c0, cw in DCH:
                wp_sb = wpp.tile([P, KT, 512], F16, name="wp_sb", tag="wp")
                nc.sync.dma_start(
                    out=wp_sb[:, :, 0:cw],
                    in_=w_d["wp"][:, :]
                    .rearrange("p (kt n) -> p kt n", n=D)[:, :, c0 : c0 + cw],
                )
                wp_tiles.append(wp_sb)

            # ---------------- attention, head-serial ----------------
            # Scores for a (head, k-tile) fill one [128, TOK] PSUM tile
            # (two 512-col matmuls) and take ONE wide exp — fewer, larger
            # ACT instructions pace the whole attention phase. attV trails
            # one k-tile behind; normalize for head h is deferred into
            # head h+1's loop so the PE never waits on the recip chain.
            pending_norm = []

            def emit_normalize(hh, rinv16, ar):
                p, hoff = hh // 2, hh % 2
                for ci, (c0, cw) in enumerate(QCH):
                    bc = psQ.tile([P, 512], F32, name="bc", tag="psQ")
                    nc.tensor.matmul(
                        bc[0:DH, 0:cw],
                        lhsT=scl_sb[0:1, 0:DH],
                        rhs=rinv16[0:1, c0 : c0 + cw],
                        start=True,
                        stop=True,
                    )
                    nc.vector.tensor_mul(
                        out=ao_sb[
                            hoff * DH : (hoff + 1) * DH, p, c0 : c0 + cw
                        ],
                        in0=ar[0:DH, c0 : c0 + cw],
                        in1=bc[0:DH, 0:cw],
                    )

            norm_kb = min(3, MT - 1)
            qk_kbs = [min(2 + 3 * i, MT - 1) for i in range(NCH)]
            for hh in range(H):
                p, hoff = hh // 2, hh % 2
                base = hoff * DH
                o_t = [
                    psO.tile([DH + 1, 512], F32, name="o_t", tag="psO")
                    for _ in QCH
                ]
                emit_next_qk = p + 1 < NPAIR
                if hoff == 0 and emit_next_qk:
                    next_tiles = emit_qk_dma2(p + 1)

                def emit_attv(kb, ee):
                    for ci, (c0, cw) in enumerate(QCH):
                        nc.tensor.matmul(
                            o_t[ci][:, 0:cw],
                            lhsT=v_sb[
                                :, kb, hh * (DH + 1) : (hh + 1) * (DH + 1)
                            ],
                            rhs=ee[:, c0 : c0 + cw],
                            start=(kb == 0),
                            stop=(kb == MT - 1),
                            skip_group_check=True,
                        )

                # attV trails the scores/exp by TWO k-tiles: its exp input
                # is then always long finished, so the in-order PE queue
                # never stalls mid-block and ACT paces at its own rate
                AVD = min(2, MT - 1)
                e_q = []
                for kb in range(MT):
                    ps = psS.tile([P, TOK], F32, tag="psS")
                    for ci, (c0, cw) in enumerate(QCH):
                        nc.tensor.matmul(
                            ps[:, c0 : c0 + cw],
                            lhsT=kT_sb[
                                base : base + DH, p, kb * P : (kb + 1) * P
                            ],
                            rhs=qT_sb[base : base + DH, p, c0 : c0 + cw],
                            start=True,
                            stop=True,
                        )
                    ee = ebuf.tile([P, TOK], BF16, tag="E")
                    nc.scalar.activation(out=ee[:, :], in_=ps[:, :], func=EXP)
                    e_q.append(ee)
                    if kb >= AVD:
                        emit_attv(kb - AVD, e_q[kb - AVD])
                    # keep PE fed: next pair's QK between attention k-tiles
                    if emit_next_qk and kb in qk_kbs:
                        emit_qk_part2(
                            p + 1, next_tiles, hoff * NCH + qk_kbs.index(kb)
                        )
                    # previous head's normalize, deps long since ready
                    if kb == norm_kb and pending_norm:
                        emit_normalize(*pending_norm.pop(0))
                for kb in range(MT - AVD, MT):
                    emit_attv(kb, e_q[kb])

                # epilogue: evict O' (bf16, incl. denom row) freeing PSUM;
                # denominators to SBUF fp32 (reciprocal_approx_fast needs
                # fp32 and misreads PSUM); recip + bf16 cast on DVE.
                den = rpool.tile([1, TOK], F32, tag="den")
                rinv = rpool.tile([1, TOK], F32, tag="r")
                rinv16 = rpool16.tile([1, TOK], BF16, tag="r16")
                ar = araw.tile([DH + 1, TOK], BF16, tag="ar")
                last = hh == H - 1
                if last:
                    # last head: denominators first so the recip chain (the
                    # serial tail before the projection) starts asap
                    for ci, (c0, cw) in enumerate(QCH):
                        nc.vector.tensor_copy(
                            out=den[0:1, c0 : c0 + cw],
                            in_=o_t[ci][DH : DH + 1, 0:cw],
                        )
                    nc.vector.reciprocal_approx_fast(out=rinv, in_=den)
                    nc.vector.tensor_copy(out=rinv16, in_=rinv)
                for ci, (c0, cw) in enumerate(QCH):
                    nc.vector.tensor_copy(
                        out=ar[:, c0 : c0 + cw], in_=o_t[ci][:, 0:cw]
                    )
                    if not last:
                        nc.vector.tensor_copy(
                            out=den[0:1, c0 : c0 + cw],
                            in_=o_t[ci][DH : DH + 1, 0:cw],
                        )
                if not last:
                    nc.vector.reciprocal_approx_fast(out=rinv, in_=den)
                    nc.vector.tensor_copy(out=rinv16, in_=rinv)
                pending_norm.append((hh, rinv16, ar))

            # ---------------- projection: out = attout wp + bp -------
            def emit_proj_group(ci, mt, ps_p, kts):
                c0, cw = DCH[ci]
                wp_sb = wp_tiles[ci]
                for kt in kts:
                    nc.tensor.matmul(
                        ps_p[:, 0:cw],
                        lhsT=ao_sb[:, kt, mt * P : (mt + 1) * P],
                        rhs=wp_sb[:, kt, 0:cw],
                        start=(kt == 0),
                        stop=False,
                    )
                if kts and kts[-1] != KT - 1:
                    return
                nc.tensor.matmul(
                    ps_p[:, 0:cw],
                    lhsT=ones_sb[0:1, 0:P],
                    rhs=bp_sb[0:1, c0 : c0 + cw],
                    start=False,
                    stop=True,
                )
                # evict on ACT (idle after the last exp) and return the
                # result over both hardware DMA rings alternately
                o_sb = outp.tile([P, 512], F32, name="o_sb", tag="o")
                nc.scalar.copy(out=o_sb[:, 0:cw], in_=ps_p[:, 0:cw])
                (nc.sync if mt % 2 == 0 else nc.scalar).dma_start(
                    out=out_d[mt * P : (mt + 1) * P, c0 : c0 + cw],
                    in_=o_sb[:, 0:cw],
                )

            while pending_norm:
                emit_normalize(*pending_norm.pop(0))
            for ci in range(len(DCH)):
                for mt in range(MT):
                    ps_p = psS.tile([P, 512], F32, name="ps_p", tag="psS")
                    emit_proj_group(ci, mt, ps_p, list(range(KT)))

    return nc


# ---------------------------------------------------------------------------
# host-side layout prep
# ---------------------------------------------------------------------------

def host_prep_shared(w_qkv, b_qkv, w_proj, b_proj, D, H):
    """Split/retile the weights once for all cores."""
    KT = D // P
    NPAIR = H // 2

    def tile_w(w):  # [D, N] -> [P, KT*N] fp16
        N = w.shape[1]
        return np.ascontiguousarray(
            w.reshape(KT, P, N).transpose(1, 0, 2).reshape(P, KT * N)
        ).astype(np.float16)

    wq3 = w_qkv.reshape(D, H, DH, 3)
    out = {
        "wq": tile_w(np.ascontiguousarray(wq3[:, :, :, 0].reshape(D, D))),
        "wk": tile_w(np.ascontiguousarray(wq3[:, :, :, 1].reshape(D, D))),
        "wv": tile_w(np.ascontiguousarray(wq3[:, :, :, 2].reshape(D, D))),
        "wp": tile_w(np.ascontiguousarray(w_proj)),
    }
    b3 = b_qkv.reshape(H, DH, 3)
    bq = np.ascontiguousarray(b3[:, :, 0].reshape(D))
    bk = np.ascontiguousarray(b3[:, :, 1].reshape(D))
    bv = np.ascontiguousarray(b3[:, :, 2].reshape(D))
    out["bq"] = np.ascontiguousarray(bq.reshape(NPAIR, P).T).astype(np.float32)
    out["bk"] = np.ascontiguousarray(bk.reshape(NPAIR, P).T).astype(np.float32)
    out["bv"] = bv.reshape(1, D).astype(np.float16)
    out["bp"] = np.asarray(b_proj, np.float32).reshape(1, D).astype(np.float16)
    return out


def host_prep_x(x_b, TOK, D):
    """One batch element [TOK, D] -> x^T tiled [P, KT*TOK] fp16."""
    KT = D // P
    xT = np.ascontiguousarray(np.asarray(x_b, np.float32).T)  # [D, TOK]
    return np.ascontiguousarray(
        xT.reshape(KT, P, TOK).transpose(1, 0, 2).reshape(P, KT * TOK)
    ).astype(np.float16)


# ---------------------------------------------------------------------------
# entry point
# ---------------------------------------------------------------------------

_BUILT = {}


def _get_nc(TOK, D, H, att_scale):
    key = (TOK, D, H, att_scale)
    if key not in _BUILT:
        nc = bacc.Bacc(
            "TRN2",
            target_bir_lowering=False,
            debug=False,
            dynamic_dma_scratch_size=512,
        )
        build(nc, TOK, D, H, att_scale)
        nc.compile()
        nc.finalize()
        _BUILT[key] = nc
    return _BUILT[key]


def kernel(x, w_qkv, b_qkv, w_proj, b_proj):
    from concourse.bass_utils import run_bass_kernel_spmd

    x = np.asarray(x, np.float32)
    B, TOK, D = x.shape
    H = H_FULL
    shared = host_prep_shared(
        np.asarray(w_qkv, np.float32),
        np.asarray(b_qkv, np.float32),
        np.asarray(w_proj, np.float32),
        np.asarray(b_proj, np.float32),
        D,
        H,
    )
    in_maps = []
    for b in range(B):
        m = dict(shared)
        m["x"] = host_prep_x(x[b], TOK, D)
        in_maps.append(m)

    nc = _get_nc(TOK, D, H, ATT_SCALE_FULL)
    res = run_bass_kernel_spmd(nc, in_maps, list(range(N_CORES)))
    out = np.stack([res.results[b]["out"] for b in range(B)], axis=0)
    return out.astype(np.float32)


# revision 83
# speedup vs baseline: 1.2215x; 1.0256x over previous
"""Multi-head attention kernel for Trainium2 (Bass/Tile), 8 NeuronCores.

Problem: nn_MultiHeadAttention
  x [8, 1024, 1024] f32, w_qkv [1024, 3072], b_qkv [3072],
  w_proj [1024, 1024], b_proj [1024]  ->  out [8, 1024, 1024]

  qkv = x @ w_qkv + b_qkv ; split (h, d, 3) interleaved on last dim
  score = q k^T per (b, h);  att = softmax(score, -1) / sqrt(1024)
  out = (att @ v) reshaped @ w_proj + b_proj

Sharding: data-parallel over batch. Each of the 8 cores runs the full
MHA for one batch element; no collectives. Host pre-transposes x and
pre-splits w_qkv so the device program is pure matmul + softmax.

Perf design (measured 686us -> 306us on HW, rel err 3.5e-3):
  - all matmul operands are 2-byte (fp16 for x/w/qT/kT/ao/wp, bf16 for
    E and v): full-rate PE streams, half-size weight loads, half DMA.
    fp32 PSUM accumulation throughout.
  - attention is head-serial: scores for one (head, k-tile) fill a
    [128, TOK] PSUM tile (two 512-col matmuls, PSUM bank limit) and
    take ONE wide exp on ACT - fewer, larger ACT instructions. attV
    trails the exp by one k-tile so it never waits on exp latency.
    PSUM: 2x scores tiles (4 banks) + 1 QK/bc bank + 3 O' banks.
  - softmax denominator rides as a 65th "ones" column of v; normalize
    uses reciprocal_approx_fast (SBUF-staged - it misreads PSUM on HW),
    a PE outer-product broadcast, and one DVE multiply, deferred by one
    head so the PE never waits on the reciprocal chain.
  - QK projection for pair p+1 and the wp prefetch are interleaved
    into the attention loop; x is split across the two hardware DMA
    rings (sync + scalar/ACT). gpsimd's software DGE corrupts strided
    DMA patterns and must only carry contiguous transfers, and its
    tensor ops (no PSUM access, ~4ns/elem) are avoided.

Device-side math per core (layouts chosen so no on-device transpose is
ever needed):
  v  = x wv + bv     [tok, (h,d)]  + ones-column per head -> v_aug
  qT = (x wq)^T + bq [(h,d), tok]
  kT = (x wk)^T + bk
  per head: S^T[k,q] = kT-slice.T @ qT-slice; E = exp(S^T) (bf16)
            O'^T[0:64,q], O'^T[64,q] = sum_k E   (v_aug ones column)
            ao^T = O'[0:64] * (att_scale / O'[64])
  out = ao^T.T @ wp + bp   (biases via ones outer-product matmuls)
"""

import os

os.environ.setdefault("MYCRO_LOCAL_CACHE", "1")

import numpy as np

import concourse.bass as bass
import concourse.tile as tile
from concourse import bacc, mybir

P = 128
DH = 64  # head dim
F32 = mybir.dt.float32
F16 = mybir.dt.float16
BF16 = mybir.dt.bfloat16

# full-problem constants
B_FULL = 8
TOK_FULL = 1024
D_FULL = 1024
H_FULL = 16
ATT_SCALE_FULL = 1.0 / 32.0  # 1/sqrt(1024), applied after softmax
N_CORES = 8


def _chunks(total, step=512):
    return [(s, min(step, total - s)) for s in range(0, total, step)]


def build(nc, TOK, D, H, att_scale):
    """Emit the one-core MHA program (one batch element).

    DRAM inputs (host pre-laid-out, fp16 unless noted):
      x        [P, KT*TOK]   [p, kt, t] = x[t, kt*P + p]   (x^T, kt-tiled)
      wq/wk/wv/wp [P, KT*D]  [p, kt, n] = w[kt*P + p, n]
      bq/bk    [P, NPAIR] f32  [p, m] = b[m*P + p]
      bv/bp    [1, D]
    Output: out [TOK, D] f32
    """
    assert D == H * DH and D % P == 0 and TOK % P == 0 and H % 2 == 0
    KT = D // P       # contraction tiles over the model dim
    MT = TOK // P     # token (and k) tiles
    NPAIR = H // 2    # head pairs (== D // P)
    VW = H * (DH + 1)  # v_aug row width: per head [v | 1]
    EXP = mybir.ActivationFunctionType.Exp
    QCH = _chunks(TOK, 512)   # q chunks (PSUM bank = 512 fp32)
    DCH = _chunks(D, 512)     # model-dim chunks

    x_d = nc.dram_tensor("x", [P, KT * TOK], F16, kind="ExternalInput")
    w_d = {}
    for nm in ("wq", "wk", "wv", "wp"):
        w_d[nm] = nc.dram_tensor(nm, [P, KT * D], F16, kind="ExternalInput")

    bq_d = nc.dram_tensor("bq", [P, NPAIR], F32, kind="ExternalInput")
    bk_d = nc.dram_tensor("bk", [P, NPAIR], F32, kind="ExternalInput")
    # bv is folded into bp on the host: softmax rows sum to 1, so
    # att @ (x wv + bv) == att @ (x wv) + bv, and the projection is
    # linear => bp' = att_scale * bv @ wp + bp (exact identity)
    bp_d = nc.dram_tensor("bp", [1, D], F16, kind="ExternalInput")
    out_d = nc.dram_tensor("out", [TOK, D], F32, kind="ExternalOutput")

    with tile.TileContext(nc) as tc:
        with (
            tc.tile_pool(name="sing", bufs=1) as sing,
            tc.tile_pool(name="psS", bufs=2, space="PSUM") as psS,
            tc.tile_pool(name="psQ", bufs=1, space="PSUM") as psQ,
            tc.tile_pool(name="psO", bufs=3, space="PSUM") as psO,
            tc.tile_pool(name="ebuf", bufs=6) as ebuf,
            tc.tile_pool(name="araw", bufs=6) as araw,
            tc.tile_pool(name="rpool", bufs=2) as rpool,
            tc.tile_pool(name="rpool16", bufs=2) as rpool16,
            tc.tile_pool(name="wqk", bufs=4) as wqk,
            tc.tile_pool(name="wpp", bufs=2) as wpp,
            tc.tile_pool(name="outp", bufs=2) as outp,
        ):
            # ---------------- persistent SBUF ----------------
            # memset targets f32; 16-bit constants made via cast copies
            cst_sb = sing.tile([1, P + DH], F32, tag="cst")
            nc.vector.memset(cst_sb[:, 0:P], 1.0)
            nc.vector.memset(cst_sb[:, P : P + DH], att_scale)
            ones_sb = sing.tile([1, P], F16, tag="ones")
            nc.vector.tensor_copy(out=ones_sb, in_=cst_sb[:, 0:P])
            scl_sb = sing.tile([1, DH], BF16, tag="scl")
            nc.vector.tensor_copy(out=scl_sb, in_=cst_sb[:, P : P + DH])
            vones_sb = sing.tile([P, MT * H], F32, tag="vones")
            nc.vector.memset(vones_sb, 1.0)

            # pair-0 qk weights first (small, they gate the first matmuls
            # together with x's first piece), then x in kt-granular pieces
            # alternating over the two hardware DMA rings (sync + scalar):
            # the first QK matmuls start as soon as piece 0 lands. gpsimd's
            # software DGE corrupts strided patterns - keep it off DMA.
            qk_tiles0 = {}
            for wi, wname in enumerate(("wq", "wk")):
                w_sb = wqk.tile([P, KT, P], F16, name="w_sb", tag="w" + wname)
                (nc.sync if wi == 0 else nc.scalar).dma_start(
                    out=w_sb[:, :, :],
                    in_=w_d[wname][:, :].rearrange("p (kt n) -> p kt n", n=D)[
                        :, :, 0:P
                    ],
                )
                qk_tiles0[wname] = w_sb

            XS = max(1, KT // 4)  # kts per x piece
            NXP = (KT + XS - 1) // XS
            x_t = []
            for j in range(NXP):
                k0, k1 = j * XS, min((j + 1) * XS, KT)
                xt = sing.tile(
                    [P, (k1 - k0) * TOK], F16, name="x_t", tag=f"x{j}"
                )
                (nc.sync if j % 2 == 0 else nc.scalar).dma_start(
                    out=xt, in_=x_d[:, k0 * TOK : k1 * TOK]
                )
                x_t.append(xt[:, :].rearrange("p (kt t) -> p kt t", t=TOK))

            def x3(kt):
                return x_t[kt // XS][:, kt % XS, :]

            bq_sb = sing.tile([P, NPAIR], F32, tag="bq")
            nc.sync.dma_start(out=bq_sb, in_=bq_d[:, :])
            bk_sb = sing.tile([P, NPAIR], F32, tag="bk")
            nc.sync.dma_start(out=bk_sb, in_=bk_d[:, :])
            bp_sb = sing.tile([1, D], F16, tag="bp")
            nc.sync.dma_start(out=bp_sb, in_=bp_d[:, :])

            v_sb = sing.tile([P, MT, VW], BF16, tag="v")     # v_aug
            # ones columns (denominator accumulators), cast f32->bf16
            nc.vector.tensor_copy(
                out=v_sb[:, :, :]
                .rearrange("p m (h e) -> p m h e", e=DH + 1)[:, :, :, DH],
                in_=vones_sb[:, :].rearrange("p (m h) -> p m h", h=H),
            )
            qT_sb = sing.tile([P, NPAIR, TOK], F16, tag="qT")
            kT_sb = sing.tile([P, NPAIR, TOK], F16, tag="kT")
            ao_sb = sing.tile([P, NPAIR, TOK], F16, tag="ao")  # attout^T

            # ---------------- QK projection helpers ----------------
            # part 0/1 = wq chunks, part 2/3 = wk chunks (when NCH==2).
            NCH = len(QCH)

            def emit_qk_dma2(pp):
                tiles = {}
                for wname in ("wq", "wk"):
                    w_sb = wqk.tile([P, KT, P], F16, name="w_sb", tag="w" + wname)
                    src = w_d[wname][:, :].rearrange("p (kt n) -> p kt n", n=D)[
                        :, :, pp * P : (pp + 1) * P
                    ]
                    nc.sync.dma_start(out=w_sb[:, :, :], in_=src)
                    tiles[wname] = w_sb
                return tiles

            def emit_qk_part2(pp, tiles, part):
                wname, dst_sb, b_sb = (
                    ("wq", qT_sb, bq_sb) if part < NCH else ("wk", kT_sb, bk_sb)
                )
                c0, cw = QCH[part % NCH]
                w_sb = tiles[wname]
                ps_q = psQ.tile([P, 512], F32, name="ps_q", tag="psQ")
                for kt in range(KT):
                    nc.tensor.matmul(
                        ps_q[:, 0:cw],
                        lhsT=w_sb[:, kt, :],
                        rhs=x3(kt)[:, c0 : c0 + cw],
                        start=(kt == 0),
                        stop=(kt == KT - 1),
                    )
                # bias-add eviction on ACT (Identity shares exp's table, so
                # no table swaps); keeps the Vector queue free for the
                # PSUM-releasing attention eviction copies
                nc.scalar.activation(
                    out=dst_sb[:, pp, c0 : c0 + cw],
                    in_=ps_q[:, 0:cw],
                    func=mybir.ActivationFunctionType.Identity,
                    bias=b_sb[:, pp : pp + 1],
                )

            # pair-0 QK first: gated only by x + a small weight slice, it
            # gives the PE work while the (larger) wv transfer is in flight
            for part in range(2 * NCH):
                emit_qk_part2(0, qk_tiles0, part)

            # ---------------- V phase: v = x wv + bv (natural) ----
            with tc.tile_pool(name="wvp", bufs=2) as wvp:
                for ci, (c0, cw) in enumerate(DCH):
                    wv_sb = wvp.tile([P, KT, 512], F16, tag="wv")
                    (nc.scalar if ci == 0 else nc.sync).dma_start(
                        out=wv_sb[:, :, 0:cw],
                        in_=w_d["wv"][:, :]
                        .rearrange("p (kt n) -> p kt n", n=D)[:, :, c0 : c0 + cw],
                    )
                    for mt in range(MT):
                        ps_v = psS.tile([P, 512], F32, tag="psS")
                        for kt in range(KT):
                            nc.tensor.matmul(
                                ps_v[:, 0:cw],
                                lhsT=x3(kt)[:, mt * P : (mt + 1) * P],
                                rhs=wv_sb[:, kt, 0:cw],
                                start=(kt == 0),
                                stop=(kt == KT - 1),
                            )
                        # scatter heads into v_aug (65-stride)
                        nh = cw // DH
                        h0 = c0 // DH
                        nc.vector.tensor_copy(
                            out=v_sb[:, mt, :]
                            .rearrange("p (h e) -> p h e", e=DH + 1)[
                                :, h0 : h0 + nh, 0:DH
                            ],
                            in_=ps_v[:, 0:cw].rearrange(
                                "p (h d) -> p h d", d=DH
                            ),
                        )

            # prefetch projection weights so they land long before the
            # projection phase starts
            wp_tiles = []
            for c0, cw in DCH:
                wp_sb = wpp.tile([P, KT, 512], F16, name="wp_sb", tag="wp")
                nc.sync.dma_start(
                    out=wp_sb[:, :, 0:cw],
                    in_=w_d["wp"][:, :]
                    .rearrange("p (kt n) -> p kt n", n=D)[:, :, c0 : c0 + cw],
                )
                wp_tiles.append(wp_sb)

            # ---------------- attention, head-serial ----------------
            # Scores for a (head, k-tile) fill one [128, TOK] PSUM tile
            # (two 512-col matmuls) and take ONE wide exp — fewer, larger
            # ACT instructions pace the whole attention phase. attV trails
            # one k-tile behind; normalize for head h is deferred into
            # head h+1's loop so the PE never waits on the recip chain.
            pending_norm = []

            def emit_normalize(hh, rinv16, ar):
                p, hoff = hh // 2, hh % 2
                for ci, (c0, cw) in enumerate(QCH):
                    bc = psQ.tile([P, 512], F32, name="bc", tag="psQ")
                    nc.tensor.matmul(
                        bc[0:DH, 0:cw],
                        lhsT=scl_sb[0:1, 0:DH],
                        rhs=rinv16[0:1, c0 : c0 + cw],
                        start=True,
                        stop=True,
                    )
                    nc.vector.tensor_mul(
                        out=ao_sb[
                            hoff * DH : (hoff + 1) * DH, p, c0 : c0 + cw
                        ],
                        in0=ar[0:DH, c0 : c0 + cw],
                        in1=bc[0:DH, 0:cw],
                    )

            norm_kb = min(3, MT - 1)
            qk_kbs = [min(2 + 3 * i, MT - 1) for i in range(NCH)]
            for hh in range(H):
                p, hoff = hh // 2, hh % 2
                base = hoff * DH
                o_t = [
                    psO.tile([DH + 1, 512], F32, name="o_t", tag="psO")
                    for _ in QCH
                ]
                emit_next_qk = p + 1 < NPAIR
                if hoff == 0 and emit_next_qk:
                    next_tiles = emit_qk_dma2(p + 1)

                def emit_attv(kb, ee):
                    for ci, (c0, cw) in enumerate(QCH):
                        nc.tensor.matmul(
                            o_t[ci][:, 0:cw],
                            lhsT=v_sb[
                                :, kb, hh * (DH + 1) : (hh + 1) * (DH + 1)
                            ],
                            rhs=ee[:, c0 : c0 + cw],
                            start=(kb == 0),
                            stop=(kb == MT - 1),
                            skip_group_check=True,
                        )

                # attV trails the scores/exp by TWO k-tiles: its exp input
                # is then always long finished, so the in-order PE queue
                # never stalls mid-block and ACT paces at its own rate
                AVD = min(2, MT - 1)
                e_q = []
                for kb in range(MT):
                    ps = psS.tile([P, TOK], F32, tag="psS")
                    for ci, (c0, cw) in enumerate(QCH):
                        nc.tensor.matmul(
                            ps[:, c0 : c0 + cw],
                            lhsT=kT_sb[
                                base : base + DH, p, kb * P : (kb + 1) * P
                            ],
                            rhs=qT_sb[base : base + DH, p, c0 : c0 + cw],
                            start=True,
                            stop=True,
                        )
                    ee = ebuf.tile([P, TOK], BF16, tag="E")
                    nc.scalar.activation(out=ee[:, :], in_=ps[:, :], func=EXP)
                    e_q.append(ee)
                    if kb >= AVD:
                        emit_attv(kb - AVD, e_q[kb - AVD])
                    # keep PE fed: next pair's QK between attention k-tiles
                    if emit_next_qk and kb in qk_kbs:
                        emit_qk_part2(
                            p + 1, next_tiles, hoff * NCH + qk_kbs.index(kb)
                        )
                    # previous head's normalize, deps long since ready
                    if kb == norm_kb and pending_norm:
                        emit_normalize(*pending_norm.pop(0))
                for kb in range(MT - AVD, MT):
                    emit_attv(kb, e_q[kb])

                # epilogue: evict O' (bf16, incl. denom row) freeing PSUM;
                # denominators to SBUF fp32 (reciprocal_approx_fast needs
                # fp32 and misreads PSUM); recip + bf16 cast on DVE.
                den = rpool.tile([1, TOK], F32, tag="den")
                rinv = rpool.tile([1, TOK], F32, tag="r")
                rinv16 = rpool16.tile([1, TOK], BF16, tag="r16")
                ar = araw.tile([DH + 1, TOK], BF16, tag="ar")
                last = hh == H - 1
                if last:
                    # last head: denominators first so the recip chain (the
                    # serial tail before the projection) starts asap
                    for ci, (c0, cw) in enumerate(QCH):
                        nc.vector.tensor_copy(
                            out=den[0:1, c0 : c0 + cw],
                            in_=o_t[ci][DH : DH + 1, 0:cw],
                        )
                    nc.vector.reciprocal_approx_fast(out=rinv, in_=den)
                    nc.vector.tensor_copy(out=rinv16, in_=rinv)
                for ci, (c0, cw) in enumerate(QCH):
                    nc.vector.tensor_copy(
                        out=ar[:, c0 : c0 + cw], in_=o_t[ci][:, 0:cw]
                    )
                    if not last:
                        nc.vector.tensor_copy(
                            out=den[0:1, c0 : c0 + cw],
                            in_=o_t[ci][DH : DH + 1, 0:cw],
                        )
                if not last:
                    nc.vector.reciprocal_approx_fast(out=rinv, in_=den)
                    nc.vector.tensor_copy(out=rinv16, in_=rinv)
                pending_norm.append((hh, rinv16, ar))

            # ---------------- projection: out = attout wp + bp -------
            def emit_proj_group(ci, mt, ps_p, kts):
                c0, cw = DCH[ci]
                wp_sb = wp_tiles[ci]
                for kt in kts:
                    nc.tensor.matmul(
                        ps_p[:, 0:cw],
                        lhsT=ao_sb[:, kt, mt * P : (mt + 1) * P],
                        rhs=wp_sb[:, kt, 0:cw],
                        start=(kt == 0),
                        stop=False,
                    )
                if kts and kts[-1] != KT - 1:
                    return
                nc.tensor.matmul(
                    ps_p[:, 0:cw],
                    lhsT=ones_sb[0:1, 0:P],
                    rhs=bp_sb[0:1, c0 : c0 + cw],
                    start=False,
                    stop=True,
                )
                # evict on ACT (idle after the last exp) and return the
                # result over both hardware DMA rings alternately
                o_sb = outp.tile([P, 512], F32, name="o_sb", tag="o")
                nc.scalar.copy(out=o_sb[:, 0:cw], in_=ps_p[:, 0:cw])
                (nc.sync if mt % 2 == 0 else nc.scalar).dma_start(
                    out=out_d[mt * P : (mt + 1) * P, c0 : c0 + cw],
                    in_=o_sb[:, 0:cw],
                )

            while pending_norm:
                emit_normalize(*pending_norm.pop(0))
            for ci in range(len(DCH)):
                for mt in range(MT):
                    ps_p = psS.tile([P, 512], F32, name="ps_p", tag="psS")
                    emit_proj_group(ci, mt, ps_p, list(range(KT)))

    return nc


# ---------------------------------------------------------------------------
# host-side layout prep
# ---------------------------------------------------------------------------

def host_prep_shared(w_qkv, b_qkv, w_proj, b_proj, D, H, att_scale):
    """Split/retile the weights once for all cores."""
    KT = D // P
    NPAIR = H // 2

    def tile_w(w):  # [D, N] -> [P, KT*N] fp16
        N = w.shape[1]
        return np.ascontiguousarray(
            w.reshape(KT, P, N).transpose(1, 0, 2).reshape(P, KT * N)
        ).astype(np.float16)

    wq3 = w_qkv.reshape(D, H, DH, 3)
    out = {
        "wq": tile_w(np.ascontiguousarray(wq3[:, :, :, 0].reshape(D, D))),
        "wk": tile_w(np.ascontiguousarray(wq3[:, :, :, 1].reshape(D, D))),
        "wv": tile_w(np.ascontiguousarray(wq3[:, :, :, 2].reshape(D, D))),
        "wp": tile_w(np.ascontiguousarray(w_proj)),
    }
    b3 = b_qkv.reshape(H, DH, 3)
    bq = np.ascontiguousarray(b3[:, :, 0].reshape(D))
    bk = np.ascontiguousarray(b3[:, :, 1].reshape(D))
    bv = np.ascontiguousarray(b3[:, :, 2].reshape(D))
    out["bq"] = np.ascontiguousarray(bq.reshape(NPAIR, P).T).astype(np.float32)
    out["bk"] = np.ascontiguousarray(bk.reshape(NPAIR, P).T).astype(np.float32)
    # softmax rows sum to 1 => bv contributes att_scale * bv @ wp to the
    # output, folded into the projection bias here (exact identity)
    bp_f = np.asarray(b_proj, np.float64) + att_scale * (
        bv.astype(np.float64) @ np.asarray(w_proj, np.float64)
    )
    out["bp"] = bp_f.reshape(1, D).astype(np.float32).astype(np.float16)
    return out


def host_prep_x(x_b, TOK, D):
    """One batch element [TOK, D] -> x^T tiled [P, KT*TOK] fp16."""
    KT = D // P
    xT = np.ascontiguousarray(np.asarray(x_b, np.float32).T)  # [D, TOK]
    return np.ascontiguousarray(
        xT.reshape(KT, P, TOK).transpose(1, 0, 2).reshape(P, KT * TOK)
    ).astype(np.float16)


# ---------------------------------------------------------------------------
# entry point
# ---------------------------------------------------------------------------

_BUILT = {}


def _get_nc(TOK, D, H, att_scale):
    key = (TOK, D, H, att_scale)
    if key not in _BUILT:
        nc = bacc.Bacc(
            "TRN2",
            target_bir_lowering=False,
            debug=False,
            dynamic_dma_scratch_size=512,
        )
        build(nc, TOK, D, H, att_scale)
        nc.compile()
        nc.finalize()
        _BUILT[key] = nc
    return _BUILT[key]


def kernel(x, w_qkv, b_qkv, w_proj, b_proj):
    from concourse.bass_utils import run_bass_kernel_spmd

    x = np.asarray(x, np.float32)
    B, TOK, D = x.shape
    H = H_FULL
    shared = host_prep_shared(
        np.asarray(w_qkv, np.float32),
        np.asarray(b_qkv, np.float32),
        np.asarray(w_proj, np.float32),
        np.asarray(b_proj, np.float32),
        D,
        H,
        ATT_SCALE_FULL,
    )
    in_maps = []
    for b in range(B):
        m = dict(shared)
        m["x"] = host_prep_x(x[b], TOK, D)
        in_maps.append(m)

    nc = _get_nc(TOK, D, H, ATT_SCALE_FULL)
    res = run_bass_kernel_spmd(nc, in_maps, list(range(N_CORES)))
    out = np.stack([res.results[b]["out"] for b in range(B)], axis=0)
    return out.astype(np.float32)
